# revision 19
# baseline (speedup 1.0000x reference)
"""Trainium2 Bass kernel for MHA cross-attention (nn_MHACross).

Sharding: 8 cores = 2 batches x 4 head-groups (2 heads each).
Each core computes, for its (batch b, head group g):
    q = x[b] @ Wq[g].T ; k,v = xmel[b] @ Wkv[g].T ; RoPE(q, k) (scale folded
    into host-side cos/sin tables); per head scores^T = k_r @ q_r^T;
    p = exp(scores) with no max subtraction (scores are O(6), safe in fp32);
    unnormalized out2 = v^T @ p and Z = ones^T @ p on the PE; normalize by
    1/Z; y_partial = attn @ Wout[:, g].T.  Host sums the 4 partial y (bf16)
    per batch in fp32.

v2 structure (vs v1):
  - k-projection first; DMA issue order matches consumption order
    (xmel s-chunk-major on sync queue, x split across scalar/vector
    queues, weights+cos/sin on gpsimd queue) to cut PE lead-in.
  - cos/sin tables in bf16 (half the DMA bytes).
  - RoPE: half-swap on scalar, swp*=sin on gpsimd, cos-mul+add on DVE
    (balances the projection-phase pipeline below the PE rate).
  - softmax 1/Z: PSUM -> DVE copy -> DVE reciprocal -> gpsimd
    partition_broadcast (no DRAM bounce).
  - y output in bf16; per-(128-row, DIM) DMA per tile.
  - attention emitted as sc(block i+1, g) / zav(block i, g) interleave so
    the PE never waits on the scalar engine's exp.
"""
import sys
sys.path.insert(0, '/opt/trn_rl_repo')
import numpy as np

DIM = 1024
NHEADS = 8
HD = 128          # head dim
HPC = 2           # heads per core
NG = 4            # head groups (cores per batch)
B, T, S = 2, 2048, 3000
NKT = DIM // 128  # contraction tiles
ROPE_BASE = 10000.0
CW = 512          # T-chunk width
PAIR = 2 * CW     # paired chunk width for exp

_cache = {}


def _ceil_div(a, b):
    return (a + b - 1) // b


def build_nc(T=T, S=S):
    from concourse import bacc, mybir
    from concourse.tile import TileContext

    f32 = mybir.dt.float32
    bf16 = mybir.dt.bfloat16

    nc = bacc.Bacc("TRN2", target_bir_lowering=False, debug=False, num_devices=8)

    xT = nc.dram_tensor("xT", [DIM, T], bf16, kind="ExternalInput")
    xmelT = nc.dram_tensor("xmelT", [DIM, S], bf16, kind="ExternalInput")
    WqT = nc.dram_tensor("WqT", [128, NKT * HPC * HD], bf16, kind="ExternalInput")
    WkT = nc.dram_tensor("WkT", [128, NKT * HPC * HD], bf16, kind="ExternalInput")
    WvT = nc.dram_tensor("WvT", [128, NKT * HPC * HD], bf16, kind="ExternalInput")
    WoT = nc.dram_tensor("WoT", [HPC * HD, DIM], bf16, kind="ExternalInput")
    cosq = nc.dram_tensor("cosq", [HD, T], bf16, kind="ExternalInput")
    sinq = nc.dram_tensor("sinq", [HD, T], bf16, kind="ExternalInput")
    cosk = nc.dram_tensor("cosk", [HD, S], bf16, kind="ExternalInput")
    sink = nc.dram_tensor("sink", [HD, S], bf16, kind="ExternalInput")
    y = nc.dram_tensor("y", [T, DIM], bf16, kind="ExternalOutput")

    n_tc = _ceil_div(T, CW)
    n_st = _ceil_div(S, 128)
    s_chunks = [(i * 512, min(512, S - i * 512)) for i in range(_ceil_div(S, 512))]
    t_chunks = [(i * CW, min(CW, T - i * CW)) for i in range(n_tc)]
    t_pairs = [t_chunks[i:i + 2] for i in range(0, n_tc, 2)]
    G = (n_st + 3) // 4   # st-group size: 4 groups per (pair, head) block

    with TileContext(nc) as tc:
        with tc.tile_pool(name="wpool", bufs=1) as wp, \
             tc.tile_pool(name="persist", bufs=1) as pp:
            # persistent weights
            wq = wp.tile([128, NKT, HPC * HD], bf16)
            wk = wp.tile([128, NKT, HPC * HD], bf16)
            wv = wp.tile([128, NKT, HPC * HD], bf16)
            wo = []
            for h in range(HPC):
                wo_h = wp.tile([128, DIM], bf16, name=f"wo{h}", uniquify=True)
                wo.append(wo_h)
            ones = wp.tile([128, 1], bf16)
            nc.vector.memset(ones[:], 1.0)

            # persistent activations
            kT_r = [pp.tile([128, S], bf16, name=f"kT{h}", uniquify=True) for h in range(HPC)]
            qT_r = [pp.tile([128, T], bf16, name=f"qT{h}", uniquify=True) for h in range(HPC)]
            v_sb = pp.tile([128, n_st, HPC * HD], bf16)

            with tc.tile_pool(name="xmelp", bufs=NKT) as xp, \
                 tc.tile_pool(name="csP", bufs=4) as csp, \
                 tc.tile_pool(name="rtP", bufs=3) as rtp, \
                 tc.tile_pool(name="aoP", bufs=2 * HPC + 2) as aoP, \
             tc.tile_pool(name="accP", bufs=2) as accP, \
                 tc.tile_pool(name="zP", bufs=6) as zP, \
                 tc.tile_pool(name="yP", bufs=2) as yP, \
                 tc.tile_pool(name="psA", bufs=2, space="PSUM") as psA, \
                 tc.tile_pool(name="psB", bufs=2, space="PSUM") as psB, \
                 tc.tile_pool(name="psC", bufs=2, space="PSUM") as psC:

                # ---- DMA prologue ----
                # Per-queue throughput is ~110-180 GB/s, so balance the three
                # trigger queues: xmel is split even/odd-kt across sync and
                # scalar; weights+cos/sin then x-odd go on gpsimd; x-even
                # follows xmel-odd on scalar.  Everything is issued in
                # consumption order.
                NCS = len(s_chunks) + len(t_chunks)  # all cos/sin tiles stay alive
                cs_k, cs_q = [], []
                for _ in s_chunks:
                    cs_k.append((csp.tile([128, 512], bf16, name="cosk_sb", tag="cos", bufs=NCS),
                                 csp.tile([128, 512], bf16, name="sink_sb", tag="sin", bufs=NCS)))
                for _ in t_chunks:
                    cs_q.append((csp.tile([128, 512], bf16, name="cosq_sb", tag="cos", bufs=NCS),
                                 csp.tile([128, 512], bf16, name="sinq_sb", tag="sin", bufs=NCS)))

                def cs_trigger(tiles, ci_, cos_d, sin_d, chunks):
                    c0, cw = chunks[ci_]
                    nc.gpsimd.dma_start(out=tiles[ci_][0][:, :cw], in_=cos_d[:, c0:c0 + cw])
                    nc.gpsimd.dma_start(out=tiles[ci_][1][:, :cw], in_=sin_d[:, c0:c0 + cw])

                # gpsimd queue prologue: wk + first two k-side cos/sin pairs;
                # the rest is dripped into the projection loop.
                nc.gpsimd.dma_start(out=wk[:], in_=WkT[:].rearrange("p (k n) -> p k n", k=NKT))
                cs_trigger(cs_k, 0, cosk, sink, s_chunks)
                cs_trigger(cs_k, 1, cosk, sink, s_chunks)

                # xmel s-chunk-major, even kt on sync / odd kt on scalar.
                # Even-kt triggers all go up front (sync has no compute);
                # odd-kt triggers are interleaved into the projection loop so
                # the scalar engine's rope copies aren't stuck behind them.
                xm = [xp.tile([128, S], bf16, name=f"xm{kt}", uniquify=True,
                              tag="xm", bufs=NKT) for kt in range(NKT)]
                for (c0, cw) in s_chunks:
                    for kt in range(0, NKT, 2):
                        nc.sync.dma_start(out=xm[kt][:, c0:c0 + cw],
                                          in_=xmelT[kt * 128:(kt + 1) * 128, c0:c0 + cw])

                def xm_odd_triggers(ci_):
                    c0, cw = s_chunks[ci_]
                    for kt in range(1, NKT, 2):
                        nc.scalar.dma_start(out=xm[kt][:, c0:c0 + cw],
                                            in_=xmelT[kt * 128:(kt + 1) * 128, c0:c0 + cw])
                xm_odd_triggers(0)

                def proj_rope(h, c0, cw, w_sb, src, cos_sb, sin_sb, out_sl):
                    ps = psA.tile([128, 512], f32, name="prps", tag="sc", bufs=2)
                    for kt in range(NKT):
                        nc.tensor.matmul(
                            ps[:, :cw],
                            w_sb[:, kt, h * HD:(h + 1) * HD],
                            src[kt][:, c0:c0 + cw],
                            start=(kt == 0), stop=(kt == NKT - 1))
                    swp = rtp.tile([128, 512], bf16, name="swp", tag="rt", bufs=3)
                    nc.scalar.copy(swp[0:64, :cw], ps[64:128, :cw])
                    nc.scalar.copy(swp[64:128, :cw], ps[0:64, :cw])
                    nc.gpsimd.tensor_tensor(out=swp[:, :cw], in0=swp[:, :cw],
                                            in1=sin_sb[:, :cw],
                                            op=mybir.AluOpType.mult)
                    nc.vector.tensor_mul(out_sl, ps[:, :cw], cos_sb[:, :cw])
                    nc.vector.tensor_add(out_sl, out_sl, swp[:, :cw])

                # ---- k(h0)+k(h1)+v interleaved per s-chunk: PE consumption
                # (~147 GB/s of xmel) tracks DMA supply; x loads ride along
                # on the scalar queue.
                with tc.tile_pool(name="xqp", bufs=NKT) as xqp:
                    xq = [xqp.tile([128, T], bf16, name=f"xq{kt}", uniquify=True,
                                   tag="xq", bufs=NKT) for kt in range(NKT)]

                    def xq_trigger(kt):
                        eng = nc.scalar if kt % 2 == 0 else nc.gpsimd
                        eng.dma_start(out=xq[kt][:], in_=xT[kt * 128:(kt + 1) * 128, :])

                    # per-iteration DMA drip on the scalar/gpsimd queues
                    # (c-index -> list of trigger thunks)
                    drip = {
                        0: [lambda: cs_trigger(cs_k, 2, cosk, sink, s_chunks),
                            lambda: nc.gpsimd.dma_start(
                                out=wv[:], in_=WvT[:].rearrange("p (k n) -> p k n", k=NKT))],
                        1: [lambda: cs_trigger(cs_k, 3, cosk, sink, s_chunks),
                            lambda: xq_trigger(1), lambda: xq_trigger(0)],
                        2: [lambda: cs_trigger(cs_k, 4, cosk, sink, s_chunks),
                            lambda: nc.gpsimd.dma_start(
                                out=wq[:], in_=WqT[:].rearrange("p (k n) -> p k n", k=NKT)),
                            lambda: xq_trigger(3), lambda: xq_trigger(2)],
                        3: [lambda: cs_trigger(cs_k, 5, cosk, sink, s_chunks),
                            lambda: xq_trigger(5), lambda: xq_trigger(4),
                            lambda: cs_trigger(cs_q, 0, cosq, sinq, t_chunks),
                            lambda: cs_trigger(cs_q, 1, cosq, sinq, t_chunks)],
                        4: [lambda: xq_trigger(7), lambda: xq_trigger(6),
                            lambda: cs_trigger(cs_q, 2, cosq, sinq, t_chunks),
                            lambda: cs_trigger(cs_q, 3, cosq, sinq, t_chunks)],
                        5: [lambda: nc.gpsimd.dma_start(out=wo[0][:], in_=WoT[0:HD, :]),
                            lambda: nc.gpsimd.dma_start(out=wo[1][:], in_=WoT[HD:2 * HD, :])],
                    }

                    for ci_, (c0, cw) in enumerate(s_chunks):
                        if ci_ + 1 < len(s_chunks):
                            xm_odd_triggers(ci_ + 1)
                        for fn in drip.get(ci_, []):
                            fn()
                        for h in range(HPC):
                            proj_rope(h, c0, cw, wk, xm, cs_k[ci_][0], cs_k[ci_][1],
                                      kT_r[h][:, c0:c0 + cw])
                        for st in range(c0 // 128, min(n_st, _ceil_div(c0 + cw, 128))):
                            s0 = st * 128
                            scnt = min(128, S - s0)
                            vps = psB.tile([128, HPC * HD], f32, name="vps", tag="acc", bufs=2)
                            for kt in range(NKT):
                                nc.tensor.matmul(
                                    vps[:scnt, :],
                                    xm[kt][:, s0:s0 + scnt],
                                    wv[:, kt, :],
                                    start=(kt == 0), stop=(kt == NKT - 1))
                            nc.vector.tensor_copy(v_sb[:scnt, st, :], vps[:scnt, :])
                    # ---- q(h0), q(h1) ----
                    for h in range(HPC):
                        for ci_, (c0, cw) in enumerate(t_chunks):
                            proj_rope(h, c0, cw, wq, xq, cs_q[ci_][0], cs_q[ci_][1],
                                      qT_r[h][:, c0:c0 + cw])

                # ---- attention machinery ----
                with tc.tile_pool(name="pP", bufs=30) as pP:
                    blocks = {}

                    def emit_sc_exp(key):
                        pi, h, g0, gc = key
                        pair = t_pairs[pi]
                        pw = sum(cw for _, cw in pair)
                        bk = blocks.setdefault((pi, h), {"ptiles": {}})
                        for st in range(g0, g0 + gc):
                            s0 = st * 128
                            scnt = min(128, S - s0)
                            scps = psA.tile([128, PAIR], f32, name="scps", tag="sc", bufs=2)
                            for ci, (c0, cw) in enumerate(pair):
                                nc.tensor.matmul(
                                    scps[:scnt, ci * CW: ci * CW + cw],
                                    kT_r[h][:, s0:s0 + scnt],
                                    qT_r[h][:, c0:c0 + cw],
                                    start=True, stop=True,
                                    skip_group_check=True)
                            p_t = pP.tile([128, PAIR], bf16, name="p_t", tag="p", bufs=30)
                            nc.scalar.activation(p_t[:scnt, :pw], scps[:scnt, :pw],
                                                 mybir.ActivationFunctionType.Exp)
                            bk["ptiles"][st] = (p_t, scnt)

                    def emit_zav(key):
                        pi, h, g0, gc = key
                        pair = t_pairs[pi]
                        bk = blocks[(pi, h)]
                        last = (g0 + gc == n_st)
                        half = n_st // 2   # st < half: DVE accumulator; rest: PE
                        if g0 == 0:
                            bk["zps"] = [psC.tile([1, CW], f32, name="zps", tag="z", bufs=2)
                                         for _ in pair]
                            bk["o2"] = [psB.tile([128, CW], f32, name="o2ps", tag="acc", bufs=2)
                                        for _ in pair]
                            bk["zacc"] = accP.tile([128, PAIR], f32, name="zacc", tag="za", bufs=1)
                        sts = list(range(g0, g0 + gc))
                        for st in sts:
                            p_t, scnt = bk["ptiles"][st]
                            if st < half:
                                # Z partial sums on the DVE (frees PE cycles);
                                # all these tiles have scnt == 128.  The last
                                # add writes bf16 so the PE can fold it in.
                                if st == 0:
                                    p1_t, _ = bk["ptiles"][1]
                                    nc.vector.tensor_add(bk["zacc"][:], p_t[:], p1_t[:])
                                elif st == half - 1:
                                    zab = accP.tile([128, PAIR], bf16, name="zab", tag="zab", bufs=1)
                                    nc.vector.tensor_add(zab[:], bk["zacc"][:], p_t[:])
                                    bk["zacc_b"] = zab
                                elif st >= 2:
                                    nc.vector.tensor_add(bk["zacc"][:], bk["zacc"][:], p_t[:])
                                continue
                            for ci, (c0, cw) in enumerate(pair):
                                if st == half:
                                    # fold the DVE accumulator into PSUM first
                                    nc.tensor.matmul(
                                        bk["zps"][ci][:, :cw],
                                        ones[:, :],
                                        bk["zacc_b"][:, ci * CW: ci * CW + cw],
                                        start=True, stop=False)
                                nc.tensor.matmul(
                                    bk["zps"][ci][:, :cw],
                                    ones[:scnt, :],
                                    p_t[:scnt, ci * CW: ci * CW + cw],
                                    start=False, stop=(st == n_st - 1))
                        if last:
                            # 1/Z: psum -> sbuf copy, reciprocal, broadcast
                            bk["zr2"] = []
                            for ci, (c0, cw) in enumerate(pair):
                                zsb = zP.tile([1, CW], f32, name="zsb", tag="zsb", bufs=2)
                                nc.vector.tensor_copy(zsb[:, :cw], bk["zps"][ci][:, :cw])
                                nc.vector.reciprocal_approx_fast(out=zsb[:, :cw], in_=zsb[:, :cw])
                                zr2 = zP.tile([128, CW], f32, name="zr2", tag="zr2", bufs=2)
                                nc.gpsimd.partition_broadcast(zr2[:, :cw], zsb[0:1, :cw],
                                                              channels=128)
                                bk["zr2"].append(zr2)
                        for st in sts:
                            p_t, scnt = bk["ptiles"][st]
                            for ci, (c0, cw) in enumerate(pair):
                                nc.tensor.matmul(
                                    bk["o2"][ci][:, :cw],
                                    v_sb[:scnt, st, h * HD:(h + 1) * HD],
                                    p_t[:scnt, ci * CW: ci * CW + cw],
                                    start=(st == 0), stop=(st == n_st - 1))
                        if last:
                            bk["ao"] = []
                            for ci, (c0, cw) in enumerate(pair):
                                ao_h = aoP.tile([128, CW], bf16, name="ao", tag="ao", bufs=2 * HPC + 2)
                                nc.vector.tensor_mul(ao_h[:, :cw], bk["o2"][ci][:, :cw], bk["zr2"][ci][:, :cw])
                                bk["ao"].append(ao_h)

                    def emit_outproj(pi):
                        pair = t_pairs[pi]
                        for ci, (c0, cw) in enumerate(pair):
                            for tt in range(cw // 128):
                                y_sb = yP.tile([128, DIM], bf16, name="y_sb", tag="ysb", bufs=2)
                                for nn in range(DIM // 512):
                                    yps = psA.tile([128, 512], f32, name="yps", tag="sc", bufs=2)
                                    for h in range(HPC):
                                        nc.tensor.matmul(
                                            yps[:],
                                            blocks[(pi, h)]["ao"][ci][:, tt * 128:(tt + 1) * 128],
                                            wo[h][:, nn * 512:(nn + 1) * 512],
                                            start=(h == 0), stop=(h == HPC - 1))
                                    if nn == 0:
                                        nc.scalar.copy(y_sb[:, nn * 512:(nn + 1) * 512], yps[:])
                                    else:
                                        nc.vector.tensor_copy(y_sb[:, nn * 512:(nn + 1) * 512], yps[:])
                                nc.sync.dma_start(out=y[c0 + tt * 128: c0 + (tt + 1) * 128, :], in_=y_sb[:])

                    # block order; sc of block i+1 interleaves with zav of block i
                    border = [(pi, h) for pi in range(len(t_pairs)) for h in range(HPC)]
                    ngr = _ceil_div(n_st, G)
                    grp = lambda bk_, g: (bk_[0], bk_[1], g * G, min(G, n_st - g * G))

                    # prime: sc for block 0 fully (exp starts early)
                    for g in range(ngr):
                        emit_sc_exp(grp(border[0], g))

                    # steady state
                    for bi in range(len(border)):
                        nxt = border[bi + 1] if bi + 1 < len(border) else None
                        for g in range(ngr):
                            if nxt is not None:
                                emit_sc_exp(grp(nxt, g))
                            emit_zav(grp(border[bi], g))
                        pi, h = border[bi]
                        if h == HPC - 1:
                            emit_outproj(pi)

    nc.compile()
    return nc


def _host_tables(T=T, S=S):
    scale = float(HD) ** (-0.25)
    inv = 1.0 / (ROPE_BASE ** (np.arange(0, HD, 2, dtype=np.float64) / HD))  # [64]

    def tables(L):
        import ml_dtypes
        fr = np.outer(inv, np.arange(L, dtype=np.float64))  # [64, L]
        c = np.cos(fr) * scale
        s = np.sin(fr) * scale
        cos = np.concatenate([c, c], axis=0).astype(ml_dtypes.bfloat16)
        sin = np.concatenate([-s, s], axis=0).astype(ml_dtypes.bfloat16)
        return np.ascontiguousarray(cos), np.ascontiguousarray(sin)

    cosq_, sinq_ = tables(T)
    cosk_, sink_ = tables(S)
    return cosq_, sinq_, cosk_, sink_


def make_in_maps(x, xmel, Wq, Wkv, Wout):
    import ml_dtypes
    bf = ml_dtypes.bfloat16
    Bx, Tx, C = x.shape
    Sx = xmel.shape[1]
    cosq_, sinq_, cosk_, sink_ = _host_tables(Tx, Sx)

    x = np.asarray(x, dtype=np.float32)
    xmel = np.asarray(xmel, dtype=np.float32)
    Wq = np.asarray(Wq, dtype=np.float32)
    Wkv = np.asarray(Wkv, dtype=np.float32)
    Wout = np.asarray(Wout, dtype=np.float32)

    xT_b = [np.ascontiguousarray(x[b].T).astype(bf) for b in range(Bx)]
    xmelT_b = [np.ascontiguousarray(xmel[b].T).astype(bf) for b in range(Bx)]
    gsz = HPC * HD  # 256
    WqT_g, WkT_g, WvT_g, WoT_g = [], [], [], []
    for g in range(NG):
        r0 = g * gsz
        def prearr(wt):  # [DIM, gsz] -> [128, NKT*gsz], row p holds [kt, n]
            return np.ascontiguousarray(
                wt.reshape(NKT, 128, gsz).transpose(1, 0, 2).reshape(128, NKT * gsz)).astype(bf)
        WqT_g.append(prearr(Wq[r0:r0 + gsz, :].T))
        WkT_g.append(prearr(Wkv[r0:r0 + gsz, :].T))
        WvT_g.append(prearr(Wkv[DIM + r0:DIM + r0 + gsz, :].T))
        WoT_g.append(np.ascontiguousarray(Wout[:, r0:r0 + gsz].T).astype(bf))

    in_maps = []
    for c in range(Bx * NG):
        b, g = c // NG, c % NG
        in_maps.append({
            "xT": xT_b[b], "xmelT": xmelT_b[b],
            "WqT": WqT_g[g], "WkT": WkT_g[g], "WvT": WvT_g[g], "WoT": WoT_g[g],
            "cosq": cosq_, "sinq": sinq_, "cosk": cosk_, "sink": sink_,
        })
    return in_maps


def kernel(x, xmel, Wq, Wkv, Wout):
    from concourse.bass_utils import run_bass_kernel_spmd

    x = np.asarray(x, dtype=np.float32)
    xmel = np.asarray(xmel, dtype=np.float32)
    Bx, Tx, C = x.shape
    Sx = xmel.shape[1]
    assert (Bx, Tx, C, Sx) == (B, T, DIM, S)

    if "nc" not in _cache:
        _cache["nc"] = build_nc()
    nc = _cache["nc"]

    in_maps = make_in_maps(x, xmel,
                           np.asarray(Wq, dtype=np.float32),
                           np.asarray(Wkv, dtype=np.float32),
                           np.asarray(Wout, dtype=np.float32))
    res = run_bass_kernel_spmd(nc, in_maps, list(range(8)))
    out = np.zeros((B, T, DIM), dtype=np.float32)
    for c in range(8):
        b = c // NG
        out[b] += res.results[c]["y"].astype(np.float32)
    return out


# revision 25
# speedup vs baseline: 1.1214x; 1.1214x over previous
"""Trainium2 Bass kernel for MHA cross-attention (nn_MHACross).

Sharding: 8 cores = 2 batches x 4 head-groups (2 heads each).
Each core computes, for its (batch b, head group g):
    q = x[b] @ Wq[g].T ; k,v = xmel[b] @ Wkv[g].T ; RoPE(q, k) (scale folded
    into host-side cos/sin tables); per head scores^T = k_r @ q_r^T;
    p = exp(scores) with no max subtraction (scores are O(6), safe in fp32);
    unnormalized out2 = v^T @ p and Z = ones^T @ p on the PE; normalize by
    1/Z; y_partial = attn @ Wout[:, g].T.  Host sums the 4 partial y (bf16)
    per batch in fp32.

v2 structure (vs v1):
  - k-projection first; DMA issue order matches consumption order
    (xmel s-chunk-major on sync queue, x split across scalar/vector
    queues, weights+cos/sin on gpsimd queue) to cut PE lead-in.
  - cos/sin tables in bf16 (half the DMA bytes).
  - RoPE: half-swap on scalar, swp*=sin on gpsimd, cos-mul+add on DVE
    (balances the projection-phase pipeline below the PE rate).
  - softmax 1/Z: PSUM -> DVE copy -> DVE reciprocal -> gpsimd
    partition_broadcast (no DRAM bounce).
  - y output in bf16; per-(128-row, DIM) DMA per tile.
  - attention emitted as sc(block i+1, g) / zav(block i, g) interleave so
    the PE never waits on the scalar engine's exp.
"""
import sys
sys.path.insert(0, '/opt/trn_rl_repo')
import numpy as np

DIM = 1024
NHEADS = 8
HD = 128          # head dim
HPC = 2           # heads per core
NG = 4            # head groups (cores per batch)
B, T, S = 2, 2048, 3000
NKT = DIM // 128  # contraction tiles
ROPE_BASE = 10000.0
CW = 512          # T-chunk width
PAIR = 2 * CW     # paired chunk width for exp

_cache = {}


def _ceil_div(a, b):
    return (a + b - 1) // b


def build_nc(T=T, S=S):
    from concourse import bacc, mybir
    from concourse.tile import TileContext

    f32 = mybir.dt.float32
    bf16 = mybir.dt.bfloat16

    nc = bacc.Bacc("TRN2", target_bir_lowering=False, debug=False, num_devices=8)

    xT = nc.dram_tensor("xT", [DIM, T], bf16, kind="ExternalInput")
    xmelT = nc.dram_tensor("xmelT", [DIM, S], bf16, kind="ExternalInput")
    WqT = nc.dram_tensor("WqT", [128, NKT * HPC * HD], bf16, kind="ExternalInput")
    WkT = nc.dram_tensor("WkT", [128, NKT * HPC * HD], bf16, kind="ExternalInput")
    WvT = nc.dram_tensor("WvT", [128, NKT * HPC * HD], bf16, kind="ExternalInput")
    WoT = nc.dram_tensor("WoT", [HPC * HD, DIM], bf16, kind="ExternalInput")
    cosq = nc.dram_tensor("cosq", [HD, T], bf16, kind="ExternalInput")
    sinq = nc.dram_tensor("sinq", [HD, T], bf16, kind="ExternalInput")
    cosk = nc.dram_tensor("cosk", [HD, S], bf16, kind="ExternalInput")
    sink = nc.dram_tensor("sink", [HD, S], bf16, kind="ExternalInput")
    y = nc.dram_tensor("y", [T, DIM], bf16, kind="ExternalOutput")

    n_tc = _ceil_div(T, CW)
    n_st = _ceil_div(S, 128)
    s_chunks = [(i * 512, min(512, S - i * 512)) for i in range(_ceil_div(S, 512))]
    t_chunks = [(i * CW, min(CW, T - i * CW)) for i in range(n_tc)]
    t_pairs = [t_chunks[i:i + 2] for i in range(0, n_tc, 2)]
    G = (n_st + 3) // 4   # st-group size: 4 groups per (pair, head) block

    with TileContext(nc) as tc:
        with tc.tile_pool(name="wpool", bufs=1) as wp, \
             tc.tile_pool(name="persist", bufs=1) as pp:
            # persistent weights
            wq = wp.tile([128, NKT, HPC * HD], bf16)
            wk = wp.tile([128, NKT, HPC * HD], bf16)
            wv = wp.tile([128, NKT, HPC * HD], bf16)
            wo = []
            for h in range(HPC):
                wo_h = wp.tile([128, DIM], bf16, name=f"wo{h}", uniquify=True)
                wo.append(wo_h)
            ones = wp.tile([128, 1], bf16)
            nc.vector.memset(ones[:], 1.0)

            # persistent activations
            kT_r = [pp.tile([128, S], bf16, name=f"kT{h}", uniquify=True) for h in range(HPC)]
            qT_r = [pp.tile([128, T], bf16, name=f"qT{h}", uniquify=True) for h in range(HPC)]
            v_sb = pp.tile([128, n_st, HPC * HD], bf16)

            with tc.tile_pool(name="xmelp", bufs=NKT) as xp, \
                 tc.tile_pool(name="csP", bufs=4) as csp, \
                 tc.tile_pool(name="rtP", bufs=3) as rtp, \
                 tc.tile_pool(name="aoP", bufs=2 * HPC + 2) as aoP, \
             tc.tile_pool(name="accP", bufs=2) as accP, \
                 tc.tile_pool(name="zP", bufs=6) as zP, \
                 tc.tile_pool(name="yP", bufs=2) as yP, \
                 tc.tile_pool(name="psA", bufs=2, space="PSUM") as psA, \
                 tc.tile_pool(name="psB", bufs=2, space="PSUM") as psB, \
                 tc.tile_pool(name="psC", bufs=2, space="PSUM") as psC:

                # ---- DMA prologue ----
                # Per-queue throughput is ~110-180 GB/s, so balance the three
                # trigger queues: xmel is split even/odd-kt across sync and
                # scalar; weights+cos/sin then x-odd go on gpsimd; x-even
                # follows xmel-odd on scalar.  Everything is issued in
                # consumption order.
                NCS = len(s_chunks) + len(t_chunks)  # all cos/sin tiles stay alive
                cs_k, cs_q = [], []
                for _ in s_chunks:
                    cs_k.append((csp.tile([128, 512], bf16, name="cosk_sb", tag="cos", bufs=NCS),
                                 csp.tile([128, 512], bf16, name="sink_sb", tag="sin", bufs=NCS)))
                for _ in t_chunks:
                    cs_q.append((csp.tile([128, 512], bf16, name="cosq_sb", tag="cos", bufs=NCS),
                                 csp.tile([128, 512], bf16, name="sinq_sb", tag="sin", bufs=NCS)))

                def cs_trigger(tiles, ci_, cos_d, sin_d, chunks):
                    c0, cw = chunks[ci_]
                    nc.gpsimd.dma_start(out=tiles[ci_][0][:, :cw], in_=cos_d[:, c0:c0 + cw])
                    nc.gpsimd.dma_start(out=tiles[ci_][1][:, :cw], in_=sin_d[:, c0:c0 + cw])

                # gpsimd queue prologue: wk + first two k-side cos/sin pairs;
                # the rest is dripped into the projection loop.
                nc.gpsimd.dma_start(out=wk[:], in_=WkT[:].rearrange("p (k n) -> p k n", k=NKT))
                cs_trigger(cs_k, 0, cosk, sink, s_chunks)
                cs_trigger(cs_k, 1, cosk, sink, s_chunks)

                # xmel s-chunk-major, even kt on sync / odd kt on scalar.
                # Even-kt triggers all go up front (sync has no compute);
                # odd-kt triggers are interleaved into the projection loop so
                # the scalar engine's rope copies aren't stuck behind them.
                xm = [xp.tile([128, S], bf16, name=f"xm{kt}", uniquify=True,
                              tag="xm", bufs=NKT) for kt in range(NKT)]
                for (c0, cw) in s_chunks:
                    for kt in range(0, NKT, 2):
                        nc.sync.dma_start(out=xm[kt][:, c0:c0 + cw],
                                          in_=xmelT[kt * 128:(kt + 1) * 128, c0:c0 + cw])

                def xm_odd_triggers(ci_):
                    c0, cw = s_chunks[ci_]
                    for kt in range(1, NKT, 2):
                        nc.scalar.dma_start(out=xm[kt][:, c0:c0 + cw],
                                            in_=xmelT[kt * 128:(kt + 1) * 128, c0:c0 + cw])
                xm_odd_triggers(0)

                def proj_rope(h, c0, cw, w_sb, src, cos_sb, sin_sb, out_sl):
                    ps = psA.tile([128, 512], f32, name="prps", tag="sc", bufs=2)
                    for kt in range(NKT):
                        nc.tensor.matmul(
                            ps[:, :cw],
                            w_sb[:, kt, h * HD:(h + 1) * HD],
                            src[kt][:, c0:c0 + cw],
                            start=(kt == 0), stop=(kt == NKT - 1))
                    swp = rtp.tile([128, 512], bf16, name="swp", tag="rt", bufs=3)
                    nc.scalar.copy(swp[0:64, :cw], ps[64:128, :cw])
                    nc.scalar.copy(swp[64:128, :cw], ps[0:64, :cw])
                    nc.gpsimd.tensor_tensor(out=swp[:, :cw], in0=swp[:, :cw],
                                            in1=sin_sb[:, :cw],
                                            op=mybir.AluOpType.mult)
                    nc.vector.tensor_mul(out_sl, ps[:, :cw], cos_sb[:, :cw])
                    nc.vector.tensor_add(out_sl, out_sl, swp[:, :cw])

                # ---- k(h0)+k(h1)+v interleaved per s-chunk: PE consumption
                # (~147 GB/s of xmel) tracks DMA supply; x loads ride along
                # on the scalar queue.
                with tc.tile_pool(name="xqp", bufs=NKT) as xqp:
                    xq = [xqp.tile([128, T], bf16, name=f"xq{kt}", uniquify=True,
                                   tag="xq", bufs=NKT) for kt in range(NKT)]

                    def xq_trigger(kt):
                        eng = nc.scalar if kt % 2 == 0 else nc.gpsimd
                        eng.dma_start(out=xq[kt][:], in_=xT[kt * 128:(kt + 1) * 128, :])

                    # per-iteration DMA drip on the scalar/gpsimd queues
                    # (c-index -> list of trigger thunks)
                    drip = {
                        0: [lambda: cs_trigger(cs_k, 2, cosk, sink, s_chunks),
                            lambda: nc.gpsimd.dma_start(
                                out=wv[:], in_=WvT[:].rearrange("p (k n) -> p k n", k=NKT))],
                        1: [lambda: cs_trigger(cs_k, 3, cosk, sink, s_chunks),
                            lambda: xq_trigger(1), lambda: xq_trigger(0)],
                        2: [lambda: cs_trigger(cs_k, 4, cosk, sink, s_chunks),
                            lambda: nc.gpsimd.dma_start(
                                out=wq[:], in_=WqT[:].rearrange("p (k n) -> p k n", k=NKT)),
                            lambda: xq_trigger(3), lambda: xq_trigger(2)],
                        3: [lambda: cs_trigger(cs_k, 5, cosk, sink, s_chunks),
                            lambda: xq_trigger(5), lambda: xq_trigger(4),
                            lambda: cs_trigger(cs_q, 0, cosq, sinq, t_chunks),
                            lambda: cs_trigger(cs_q, 1, cosq, sinq, t_chunks)],
                        4: [lambda: xq_trigger(7), lambda: xq_trigger(6),
                            lambda: cs_trigger(cs_q, 2, cosq, sinq, t_chunks),
                            lambda: cs_trigger(cs_q, 3, cosq, sinq, t_chunks)],
                        5: [lambda: nc.gpsimd.dma_start(out=wo[0][:], in_=WoT[0:HD, :]),
                            lambda: nc.gpsimd.dma_start(out=wo[1][:], in_=WoT[HD:2 * HD, :])],
                    }

                    for ci_, (c0, cw) in enumerate(s_chunks):
                        if ci_ + 1 < len(s_chunks):
                            xm_odd_triggers(ci_ + 1)
                        for fn in drip.get(ci_, []):
                            fn()
                        for h in range(HPC):
                            proj_rope(h, c0, cw, wk, xm, cs_k[ci_][0], cs_k[ci_][1],
                                      kT_r[h][:, c0:c0 + cw])
                        for st in range(c0 // 128, min(n_st, _ceil_div(c0 + cw, 128))):
                            s0 = st * 128
                            scnt = min(128, S - s0)
                            vps = psB.tile([128, HPC * HD], f32, name="vps", tag="acc", bufs=2)
                            for kt in range(NKT):
                                nc.tensor.matmul(
                                    vps[:scnt, :],
                                    xm[kt][:, s0:s0 + scnt],
                                    wv[:, kt, :],
                                    start=(kt == 0), stop=(kt == NKT - 1))
                            nc.vector.tensor_copy(v_sb[:scnt, st, :], vps[:scnt, :])
                    # ---- q(h0), q(h1) ----
                    for h in range(HPC):
                        for ci_, (c0, cw) in enumerate(t_chunks):
                            proj_rope(h, c0, cw, wq, xq, cs_q[ci_][0], cs_q[ci_][1],
                                      qT_r[h][:, c0:c0 + cw])

                # ---- attention machinery ----
                with tc.tile_pool(name="pP", bufs=30) as pP:
                    blocks = {}

                    def emit_sc_exp(key):
                        pi, h, g0, gc = key
                        pair = t_pairs[pi]
                        pw = sum(cw for _, cw in pair)
                        bk = blocks.setdefault((pi, h), {"ptiles": {}})
                        for st in range(g0, g0 + gc):
                            s0 = st * 128
                            scnt = min(128, S - s0)
                            scps = psA.tile([128, PAIR], f32, name="scps", tag="sc", bufs=2)
                            for ci, (c0, cw) in enumerate(pair):
                                nc.tensor.matmul(
                                    scps[:scnt, ci * CW: ci * CW + cw],
                                    kT_r[h][:, s0:s0 + scnt],
                                    qT_r[h][:, c0:c0 + cw],
                                    start=True, stop=True,
                                    skip_group_check=True)
                            p_t = pP.tile([128, PAIR], bf16, name="p_t", tag="p", bufs=30)
                            nc.scalar.activation(p_t[:scnt, :pw], scps[:scnt, :pw],
                                                 mybir.ActivationFunctionType.Exp)
                            bk["ptiles"][st] = (p_t, scnt)

                    def emit_zav(key):
                        pi, h, g0, gc = key
                        pair = t_pairs[pi]
                        bk = blocks[(pi, h)]
                        last = (g0 + gc == n_st)
                        half = n_st // 2   # st < half: DVE accumulator; rest: PE
                        if g0 == 0:
                            bk["zps"] = [psC.tile([1, CW], f32, name="zps", tag="z", bufs=2)
                                         for _ in pair]
                            bk["o2"] = [psB.tile([128, CW], f32, name="o2ps", tag="acc", bufs=2)
                                        for _ in pair]
                            bk["zacc"] = accP.tile([128, PAIR], f32, name="zacc", tag="za", bufs=1)
                        sts = list(range(g0, g0 + gc))
                        q1 = n_st // 4            # st < q1: DVE;  q1 <= st < half: gpsimd
                        for st in sts:
                            p_t, scnt = bk["ptiles"][st]
                            if st < half:
                                # Z partial sums off the PE: groups g0 on the
                                # DVE, g1 on gpsimd (separate accumulators);
                                # the final add of each chain writes bf16 so
                                # the PE can fold it into PSUM at block end.
                                # All these tiles have scnt == 128.
                                eng = nc.vector if st < q1 else nc.gpsimd
                                acc_key = "zacc" if st < q1 else "zaccg"
                                first, final = (0, q1 - 1) if st < q1 else (q1, half - 1)
                                if st == first:
                                    p1_t, _ = bk["ptiles"][st + 1]
                                    if acc_key not in bk:
                                        bk[acc_key] = accP.tile(
                                            [128, PAIR], f32, name=acc_key,
                                            tag=acc_key, bufs=1)
                                    eng.tensor_tensor(out=bk[acc_key][:], in0=p_t[:],
                                                      in1=p1_t[:], op=mybir.AluOpType.add)
                                elif st == final:
                                    zab = accP.tile([128, PAIR], bf16, name=acc_key + "b",
                                                    tag=acc_key + "b", bufs=1)
                                    eng.tensor_tensor(out=zab[:], in0=bk[acc_key][:],
                                                      in1=p_t[:], op=mybir.AluOpType.add)
                                    bk[acc_key + "_b"] = zab
                                elif st > first + 1:
                                    eng.tensor_tensor(out=bk[acc_key][:], in0=bk[acc_key][:],
                                                      in1=p_t[:], op=mybir.AluOpType.add)
                                continue
                            for ci, (c0, cw) in enumerate(pair):
                                nc.tensor.matmul(
                                    bk["zps"][ci][:, :cw],
                                    ones[:scnt, :],
                                    p_t[:scnt, ci * CW: ci * CW + cw],
                                    start=(st == half), stop=False)
                        if last:
                            # fold the DVE/gpsimd accumulators into PSUM at the
                            # very end, giving both add-chains the whole block
                            # to complete without stalling the PE.
                            for ci, (c0, cw) in enumerate(pair):
                                nc.tensor.matmul(
                                    bk["zps"][ci][:, :cw], ones[:, :],
                                    bk["zacc_b"][:, ci * CW: ci * CW + cw],
                                    start=False, stop=False)
                                nc.tensor.matmul(
                                    bk["zps"][ci][:, :cw], ones[:, :],
                                    bk["zaccg_b"][:, ci * CW: ci * CW + cw],
                                    start=False, stop=True)
                        if last:
                            # 1/Z: psum -> sbuf copy, reciprocal, broadcast
                            bk["zr2"] = []
                            for ci, (c0, cw) in enumerate(pair):
                                zsb = zP.tile([1, CW], f32, name="zsb", tag="zsb", bufs=2)
                                nc.vector.tensor_copy(zsb[:, :cw], bk["zps"][ci][:, :cw])
                                nc.vector.reciprocal_approx_fast(out=zsb[:, :cw], in_=zsb[:, :cw])
                                zr2 = zP.tile([128, CW], f32, name="zr2", tag="zr2", bufs=2)
                                nc.gpsimd.partition_broadcast(zr2[:, :cw], zsb[0:1, :cw],
                                                              channels=128)
                                bk["zr2"].append(zr2)
                        for st in sts:
                            p_t, scnt = bk["ptiles"][st]
                            for ci, (c0, cw) in enumerate(pair):
                                nc.tensor.matmul(
                                    bk["o2"][ci][:, :cw],
                                    v_sb[:scnt, st, h * HD:(h + 1) * HD],
                                    p_t[:scnt, ci * CW: ci * CW + cw],
                                    start=(st == 0), stop=(st == n_st - 1))
                        if last:
                            bk["ao"] = []
                            for ci, (c0, cw) in enumerate(pair):
                                ao_h = aoP.tile([128, CW], bf16, name="ao", tag="ao", bufs=2 * HPC + 2)
                                nc.vector.tensor_mul(ao_h[:, :cw], bk["o2"][ci][:, :cw], bk["zr2"][ci][:, :cw])
                                bk["ao"].append(ao_h)

                    def emit_outproj(pi):
                        # keep the exp stream clean: scalar only does y copies
                        # for the final pair (its exps are all done by then)
                        pair = t_pairs[pi]
                        last_pair = (pi == len(t_pairs) - 1)
                        for ci, (c0, cw) in enumerate(pair):
                            for tt in range(cw // 128):
                                y_sb = yP.tile([128, DIM], bf16, name="y_sb", tag="ysb", bufs=2)
                                for nn in range(DIM // 512):
                                    yps = psA.tile([128, 512], f32, name="yps", tag="sc", bufs=2)
                                    for h in range(HPC):
                                        nc.tensor.matmul(
                                            yps[:],
                                            blocks[(pi, h)]["ao"][ci][:, tt * 128:(tt + 1) * 128],
                                            wo[h][:, nn * 512:(nn + 1) * 512],
                                            start=(h == 0), stop=(h == HPC - 1))
                                    dst = y_sb[:, nn * 512:(nn + 1) * 512]
                                    if nn == 1:
                                        nc.vector.tensor_copy(dst, yps[:])
                                    elif last_pair:
                                        nc.scalar.copy(dst, yps[:])
                                    else:
                                        nc.vector.tensor_copy(dst, yps[:])
                                nc.sync.dma_start(out=y[c0 + tt * 128: c0 + (tt + 1) * 128, :], in_=y_sb[:])

                    # block order; sc of block i+1 interleaves with zav of block i
                    border = [(pi, h) for pi in range(len(t_pairs)) for h in range(HPC)]
                    ngr = _ceil_div(n_st, G)
                    grp = lambda bk_, g: (bk_[0], bk_[1], g * G, min(G, n_st - g * G))

                    # prime: sc for block 0 fully (exp starts early)
                    for g in range(ngr):
                        emit_sc_exp(grp(border[0], g))

                    # steady state
                    for bi in range(len(border)):
                        nxt = border[bi + 1] if bi + 1 < len(border) else None
                        for g in range(ngr):
                            if nxt is not None:
                                emit_sc_exp(grp(nxt, g))
                            emit_zav(grp(border[bi], g))
                        pi, h = border[bi]
                        if h == HPC - 1:
                            emit_outproj(pi)

    nc.compile()
    return nc


def _host_tables(T=T, S=S):
    scale = float(HD) ** (-0.25)
    inv = 1.0 / (ROPE_BASE ** (np.arange(0, HD, 2, dtype=np.float64) / HD))  # [64]

    def tables(L):
        import ml_dtypes
        fr = np.outer(inv, np.arange(L, dtype=np.float64))  # [64, L]
        c = np.cos(fr) * scale
        s = np.sin(fr) * scale
        cos = np.concatenate([c, c], axis=0).astype(ml_dtypes.bfloat16)
        sin = np.concatenate([-s, s], axis=0).astype(ml_dtypes.bfloat16)
        return np.ascontiguousarray(cos), np.ascontiguousarray(sin)

    cosq_, sinq_ = tables(T)
    cosk_, sink_ = tables(S)
    return cosq_, sinq_, cosk_, sink_


def make_in_maps(x, xmel, Wq, Wkv, Wout):
    import ml_dtypes
    bf = ml_dtypes.bfloat16
    Bx, Tx, C = x.shape
    Sx = xmel.shape[1]
    cosq_, sinq_, cosk_, sink_ = _host_tables(Tx, Sx)

    x = np.asarray(x, dtype=np.float32)
    xmel = np.asarray(xmel, dtype=np.float32)
    Wq = np.asarray(Wq, dtype=np.float32)
    Wkv = np.asarray(Wkv, dtype=np.float32)
    Wout = np.asarray(Wout, dtype=np.float32)

    xT_b = [np.ascontiguousarray(x[b].T).astype(bf) for b in range(Bx)]
    xmelT_b = [np.ascontiguousarray(xmel[b].T).astype(bf) for b in range(Bx)]
    gsz = HPC * HD  # 256
    WqT_g, WkT_g, WvT_g, WoT_g = [], [], [], []
    for g in range(NG):
        r0 = g * gsz
        def prearr(wt):  # [DIM, gsz] -> [128, NKT*gsz], row p holds [kt, n]
            return np.ascontiguousarray(
                wt.reshape(NKT, 128, gsz).transpose(1, 0, 2).reshape(128, NKT * gsz)).astype(bf)
        WqT_g.append(prearr(Wq[r0:r0 + gsz, :].T))
        WkT_g.append(prearr(Wkv[r0:r0 + gsz, :].T))
        WvT_g.append(prearr(Wkv[DIM + r0:DIM + r0 + gsz, :].T))
        WoT_g.append(np.ascontiguousarray(Wout[:, r0:r0 + gsz].T).astype(bf))

    in_maps = []
    for c in range(Bx * NG):
        b, g = c // NG, c % NG
        in_maps.append({
            "xT": xT_b[b], "xmelT": xmelT_b[b],
            "WqT": WqT_g[g], "WkT": WkT_g[g], "WvT": WvT_g[g], "WoT": WoT_g[g],
            "cosq": cosq_, "sinq": sinq_, "cosk": cosk_, "sink": sink_,
        })
    return in_maps


def kernel(x, xmel, Wq, Wkv, Wout):
    from concourse.bass_utils import run_bass_kernel_spmd

    x = np.asarray(x, dtype=np.float32)
    xmel = np.asarray(xmel, dtype=np.float32)
    Bx, Tx, C = x.shape
    Sx = xmel.shape[1]
    assert (Bx, Tx, C, Sx) == (B, T, DIM, S)

    if "nc" not in _cache:
        _cache["nc"] = build_nc()
    nc = _cache["nc"]

    in_maps = make_in_maps(x, xmel,
                           np.asarray(Wq, dtype=np.float32),
                           np.asarray(Wkv, dtype=np.float32),
                           np.asarray(Wout, dtype=np.float32))
    res = run_bass_kernel_spmd(nc, in_maps, list(range(8)))
    out = np.zeros((B, T, DIM), dtype=np.float32)
    for c in range(8):
        b = c // NG
        out[b] += res.results[c]["y"].astype(np.float32)
    return out


# revision 34
# speedup vs baseline: 1.1816x; 1.0536x over previous
"""Trainium2 Bass kernel for MHA cross-attention (nn_MHACross).

Sharding: 8 cores = 2 batches x 4 head-groups (2 heads each).
Each core computes, for its (batch b, head group g):
    q = x[b] @ Wq[g].T ; k,v = xmel[b] @ Wkv[g].T ; RoPE(q, k) (scale folded
    into host-side cos/sin tables); per head scores^T = k_r @ q_r^T;
    p = exp(scores) with no max subtraction (scores are O(6), safe in fp32);
    unnormalized out2 = v^T @ p and Z = ones^T @ p on the PE; normalize by
    1/Z; y_partial = attn @ Wout[:, g].T.  Host sums the 4 partial y (bf16)
    per batch in fp32.

v2 structure (vs v1):
  - k-projection first; DMA issue order matches consumption order
    (xmel s-chunk-major on sync queue, x split across scalar/vector
    queues, weights+cos/sin on gpsimd queue) to cut PE lead-in.
  - cos/sin tables in bf16 (half the DMA bytes).
  - RoPE: half-swap on scalar, swp*=sin on gpsimd, cos-mul+add on DVE
    (balances the projection-phase pipeline below the PE rate).
  - softmax 1/Z: PSUM -> DVE copy -> DVE reciprocal -> gpsimd
    partition_broadcast (no DRAM bounce).
  - y output in bf16; per-(128-row, DIM) DMA per tile.
  - attention emitted as sc(block i+1, g) / zav(block i, g) interleave so
    the PE never waits on the scalar engine's exp.
"""
import sys
sys.path.insert(0, '/opt/trn_rl_repo')
import numpy as np

DIM = 1024
NHEADS = 8
HD = 128          # head dim
HPC = 2           # heads per core
NG = 4            # head groups (cores per batch)
B, T, S = 2, 2048, 3000
NKT = DIM // 128  # contraction tiles
ROPE_BASE = 10000.0
CW = 512          # T-chunk width
PAIR = 2 * CW     # paired chunk width for exp

_cache = {}


def _ceil_div(a, b):
    return (a + b - 1) // b


def build_nc(T=T, S=S):
    from concourse import bacc, mybir
    from concourse.tile import TileContext

    f32 = mybir.dt.float32
    f32r = mybir.dt.float32r
    bf16 = mybir.dt.bfloat16

    nc = bacc.Bacc("TRN2", target_bir_lowering=False, debug=False, num_devices=8)

    xT = nc.dram_tensor("xT", [DIM, T], bf16, kind="ExternalInput")
    xmelT = nc.dram_tensor("xmelT", [DIM, S], bf16, kind="ExternalInput")
    WqT = nc.dram_tensor("WqT", [128, NKT * HPC * HD], bf16, kind="ExternalInput")
    WkT = nc.dram_tensor("WkT", [128, NKT * HPC * HD], bf16, kind="ExternalInput")
    WvT = nc.dram_tensor("WvT", [128, NKT * HPC * HD], bf16, kind="ExternalInput")
    WoT = nc.dram_tensor("WoT", [HPC * HD, DIM], bf16, kind="ExternalInput")
    cosq = nc.dram_tensor("cosq", [HD, T], bf16, kind="ExternalInput")
    sinq = nc.dram_tensor("sinq", [HD, T], bf16, kind="ExternalInput")
    cosk = nc.dram_tensor("cosk", [HD, S], bf16, kind="ExternalInput")
    sink = nc.dram_tensor("sink", [HD, S], bf16, kind="ExternalInput")
    y = nc.dram_tensor("y", [T, DIM], bf16, kind="ExternalOutput")

    n_tc = _ceil_div(T, CW)
    n_st = _ceil_div(S, 128)
    s_chunks = [(i * 512, min(512, S - i * 512)) for i in range(_ceil_div(S, 512))]
    t_chunks = [(i * CW, min(CW, T - i * CW)) for i in range(n_tc)]
    t_pairs = [t_chunks[i:i + 2] for i in range(0, n_tc, 2)]
    G = (n_st + 3) // 4   # st-group size: 4 groups per (pair, head) block

    with TileContext(nc) as tc:
        with tc.tile_pool(name="wpool", bufs=1) as wp, \
             tc.tile_pool(name="persist", bufs=1) as pp:
            # persistent weights
            wq = wp.tile([128, NKT, HPC * HD], bf16)
            wk = wp.tile([128, NKT, HPC * HD], bf16)
            wv = wp.tile([128, NKT, HPC * HD], bf16)
            wo = []
            for h in range(HPC):
                wo_h = wp.tile([128, DIM], bf16, name=f"wo{h}", uniquify=True)
                wo.append(wo_h)
            ones = wp.tile([128, 1], bf16)
            nc.vector.memset(ones[:], 1.0)
            onesrow = wp.tile([1, 128], bf16)
            nc.vector.memset(onesrow[:], 1.0)
            # dummy op so gpsimd's TENSOR_TENSOR ucode lib loads during the
            # DMA lead-in instead of stalling the first rope multiply (~12us)
            gpwarm = wp.tile([1, 512], bf16)
            nc.vector.memset(gpwarm[:], 0.0)
            nc.gpsimd.tensor_tensor(out=gpwarm[:], in0=gpwarm[:], in1=gpwarm[:],
                                    op=mybir.AluOpType.add)

            # persistent activations
            kT_r = [pp.tile([128, S], bf16, name=f"kT{h}", uniquify=True) for h in range(HPC)]
            qT_r = [pp.tile([128, T], bf16, name=f"qT{h}", uniquify=True) for h in range(HPC)]
            v_sb = pp.tile([128, n_st, HPC * HD], bf16)

            with tc.tile_pool(name="xmelp", bufs=NKT) as xp, \
                 tc.tile_pool(name="csP", bufs=4) as csp, \
                 tc.tile_pool(name="rtP", bufs=3) as rtp, \
                 tc.tile_pool(name="aoP", bufs=2 * HPC + 2) as aoP, \
             tc.tile_pool(name="accP", bufs=2) as accP, \
                 tc.tile_pool(name="zP", bufs=6) as zP, \
                 tc.tile_pool(name="yP", bufs=2) as yP, \
                 tc.tile_pool(name="psA", bufs=2, space="PSUM") as psA, \
                 tc.tile_pool(name="psB", bufs=2, space="PSUM") as psB, \
                 tc.tile_pool(name="psC", bufs=2, space="PSUM") as psC:

                # ---- DMA prologue ----
                # Per-queue throughput is ~110-180 GB/s, so balance the three
                # trigger queues: xmel is split even/odd-kt across sync and
                # scalar; weights+cos/sin then x-odd go on gpsimd; x-even
                # follows xmel-odd on scalar.  Everything is issued in
                # consumption order.
                NCS = len(s_chunks) + len(t_chunks)  # all cos/sin tiles stay alive
                cs_k, cs_q = [], []
                for _ in s_chunks:
                    cs_k.append((csp.tile([128, 512], bf16, name="cosk_sb", tag="cos", bufs=NCS),
                                 csp.tile([128, 512], bf16, name="sink_sb", tag="sin", bufs=NCS)))
                for _ in t_chunks:
                    cs_q.append((csp.tile([128, 512], bf16, name="cosq_sb", tag="cos", bufs=NCS),
                                 csp.tile([128, 512], bf16, name="sinq_sb", tag="sin", bufs=NCS)))

                def cs_trigger(tiles, ci_, cos_d, sin_d, chunks):
                    c0, cw = chunks[ci_]
                    nc.gpsimd.dma_start(out=tiles[ci_][0][:, :cw], in_=cos_d[:, c0:c0 + cw])
                    nc.gpsimd.dma_start(out=tiles[ci_][1][:, :cw], in_=sin_d[:, c0:c0 + cw])

                # gpsimd queue prologue: wk + first two k-side cos/sin pairs;
                # the rest is dripped into the projection loop.
                nc.gpsimd.dma_start(out=wk[:], in_=WkT[:].rearrange("p (k n) -> p k n", k=NKT))
                cs_trigger(cs_k, 0, cosk, sink, s_chunks)
                cs_trigger(cs_k, 1, cosk, sink, s_chunks)

                # xmel s-chunk-major, even kt on sync / odd kt on scalar.
                # Even-kt triggers all go up front (sync has no compute);
                # odd-kt triggers are interleaved into the projection loop so
                # the scalar engine's rope copies aren't stuck behind them.
                xm = [xp.tile([128, S], bf16, name=f"xm{kt}", uniquify=True,
                              tag="xm", bufs=NKT) for kt in range(NKT)]
                for (c0, cw) in s_chunks:
                    for kt in range(0, NKT, 2):
                        nc.sync.dma_start(out=xm[kt][:, c0:c0 + cw],
                                          in_=xmelT[kt * 128:(kt + 1) * 128, c0:c0 + cw])

                def xm_odd_triggers(ci_):
                    c0, cw = s_chunks[ci_]
                    for kt in range(1, NKT, 2):
                        nc.scalar.dma_start(out=xm[kt][:, c0:c0 + cw],
                                            in_=xmelT[kt * 128:(kt + 1) * 128, c0:c0 + cw])
                xm_odd_triggers(0)

                def proj_rope(h, c0, cw, w_sb, src, cos_sb, sin_sb, out_sl):
                    ps = psA.tile([128, 512], f32, name="prps", tag="sc", bufs=2)
                    for kt in range(NKT):
                        nc.tensor.matmul(
                            ps[:, :cw],
                            w_sb[:, kt, h * HD:(h + 1) * HD],
                            src[kt][:, c0:c0 + cw],
                            start=(kt == 0), stop=(kt == NKT - 1))
                    swp = rtp.tile([128, 512], bf16, name="swp", tag="rt", bufs=3)
                    nc.scalar.copy(swp[0:64, :cw], ps[64:128, :cw])
                    nc.scalar.copy(swp[64:128, :cw], ps[0:64, :cw])
                    nc.gpsimd.tensor_tensor(out=swp[:, :cw], in0=swp[:, :cw],
                                            in1=sin_sb[:, :cw],
                                            op=mybir.AluOpType.mult)
                    nc.vector.tensor_mul(out_sl, ps[:, :cw], cos_sb[:, :cw])
                    nc.vector.tensor_add(out_sl, out_sl, swp[:, :cw])

                # ---- k(h0)+k(h1)+v interleaved per s-chunk: PE consumption
                # (~147 GB/s of xmel) tracks DMA supply; x loads ride along
                # on the scalar queue.
                with tc.tile_pool(name="xqp", bufs=NKT) as xqp:
                    xq = [xqp.tile([128, T], bf16, name=f"xq{kt}", uniquify=True,
                                   tag="xq", bufs=NKT) for kt in range(NKT)]

                    def xq_trigger(kt):
                        eng = nc.scalar if kt % 2 == 0 else nc.gpsimd
                        eng.dma_start(out=xq[kt][:], in_=xT[kt * 128:(kt + 1) * 128, :])

                    # per-iteration DMA drip on the scalar/gpsimd queues
                    # (c-index -> list of trigger thunks)
                    drip = {
                        0: [lambda: cs_trigger(cs_k, 2, cosk, sink, s_chunks),
                            lambda: nc.gpsimd.dma_start(
                                out=wv[:], in_=WvT[:].rearrange("p (k n) -> p k n", k=NKT)),
                            lambda: xq_trigger(1), lambda: xq_trigger(0)],
                        1: [lambda: cs_trigger(cs_k, 3, cosk, sink, s_chunks),
                            lambda: xq_trigger(3), lambda: xq_trigger(2)],
                        2: [lambda: cs_trigger(cs_k, 4, cosk, sink, s_chunks),
                            lambda: nc.gpsimd.dma_start(
                                out=wq[:], in_=WqT[:].rearrange("p (k n) -> p k n", k=NKT)),
                            lambda: xq_trigger(5), lambda: xq_trigger(4)],
                        3: [lambda: cs_trigger(cs_k, 5, cosk, sink, s_chunks),
                            lambda: xq_trigger(7), lambda: xq_trigger(6),
                            lambda: cs_trigger(cs_q, 0, cosq, sinq, t_chunks),
                            lambda: cs_trigger(cs_q, 1, cosq, sinq, t_chunks)],
                        4: [lambda: cs_trigger(cs_q, 2, cosq, sinq, t_chunks),
                            lambda: cs_trigger(cs_q, 3, cosq, sinq, t_chunks)],
                        5: [lambda: nc.gpsimd.dma_start(out=wo[0][:], in_=WoT[0:HD, :]),
                            lambda: nc.gpsimd.dma_start(out=wo[1][:], in_=WoT[HD:2 * HD, :])],
                    }

                    for ci_, (c0, cw) in enumerate(s_chunks):
                        if ci_ + 1 < len(s_chunks):
                            xm_odd_triggers(ci_ + 1)
                        for fn in drip.get(ci_, []):
                            fn()
                        for h in range(HPC):
                            proj_rope(h, c0, cw, wk, xm, cs_k[ci_][0], cs_k[ci_][1],
                                      kT_r[h][:, c0:c0 + cw])
                        for st in range(c0 // 128, min(n_st, _ceil_div(c0 + cw, 128))):
                            s0 = st * 128
                            scnt = min(128, S - s0)
                            vps = psB.tile([128, HPC * HD], f32, name="vps", tag="acc", bufs=2)
                            for kt in range(NKT):
                                nc.tensor.matmul(
                                    vps[:scnt, :],
                                    xm[kt][:, s0:s0 + scnt],
                                    wv[:, kt, :],
                                    start=(kt == 0), stop=(kt == NKT - 1))
                            nc.vector.tensor_copy(v_sb[:scnt, st, :], vps[:scnt, :])
                    # ---- q(h0), q(h1) ----
                    for h in range(HPC):
                        for ci_, (c0, cw) in enumerate(t_chunks):
                            proj_rope(h, c0, cw, wq, xq, cs_q[ci_][0], cs_q[ci_][1],
                                      qT_r[h][:, c0:c0 + cw])

                # ---- attention machinery ----
                with tc.tile_pool(name="pP", bufs=30) as pP:
                    blocks = {}

                    def emit_sc_exp(key):
                        pi, h, g0, gc = key
                        pair = t_pairs[pi]
                        pw = sum(cw for _, cw in pair)
                        bk = blocks.setdefault((pi, h), {"ptiles": {}})
                        for st in range(g0, g0 + gc):
                            s0 = st * 128
                            scnt = min(128, S - s0)
                            scps = psA.tile([128, PAIR], f32, name="scps", tag="sc", bufs=2)
                            for ci, (c0, cw) in enumerate(pair):
                                nc.tensor.matmul(
                                    scps[:scnt, ci * CW: ci * CW + cw],
                                    kT_r[h][:, s0:s0 + scnt],
                                    qT_r[h][:, c0:c0 + cw],
                                    start=True, stop=True,
                                    skip_group_check=True)
                            p_t = pP.tile([128, PAIR], bf16, name="p_t", tag="p", bufs=30)
                            nc.scalar.activation(p_t[:scnt, :pw], scps[:scnt, :pw],
                                                 mybir.ActivationFunctionType.Exp)
                            bk["ptiles"][st] = (p_t, scnt)

                    def emit_zav(key):
                        pi, h, g0, gc = key
                        pair = t_pairs[pi]
                        bk = blocks[(pi, h)]
                        last = (g0 + gc == n_st)
                        half = n_st // 2   # st < half: DVE accumulator; rest: PE
                        if g0 == 0:
                            bk["zps"] = [psC.tile([1, CW], f32, name="zps", tag="z", bufs=2)
                                         for _ in pair]
                            bk["o2"] = [psB.tile([128, CW], f32, name="o2ps", tag="acc", bufs=2)
                                        for _ in pair]
                            bk["zacc"] = accP.tile([128, PAIR], f32, name="zacc", tag="za", bufs=1)
                        sts = list(range(g0, g0 + gc))
                        for st in sts:
                            p_t, scnt = bk["ptiles"][st]
                            if st < half:
                                # Z partial sums for the first half on the DVE
                                # (serial chain, ~1.2us/add, finishes well
                                # before block end); final add writes bf16 so
                                # the PE can fold it into PSUM.  All these
                                # tiles have scnt == 128.
                                if st == 0:
                                    p1_t, _ = bk["ptiles"][1]
                                    nc.vector.tensor_add(bk["zacc"][:], p_t[:], p1_t[:])
                                elif st == half - 1:
                                    zab = accP.tile([128, PAIR], bf16, name="zab",
                                                    tag="zab", bufs=1)
                                    nc.vector.tensor_add(zab[:], bk["zacc"][:], p_t[:])
                                    bk["zacc_b"] = zab
                                elif st > 1:
                                    nc.vector.tensor_add(bk["zacc"][:], bk["zacc"][:], p_t[:])
                                continue
                            for ci, (c0, cw) in enumerate(pair):
                                nc.tensor.matmul(
                                    bk["zps"][ci][:, :cw],
                                    ones[:scnt, :],
                                    p_t[:scnt, ci * CW: ci * CW + cw],
                                    start=(st == half), stop=False)
                        if last:
                            # fold the DVE accumulator in at the very end, so
                            # the add-chain has the whole block to complete
                            # without stalling the PE
                            for ci, (c0, cw) in enumerate(pair):
                                nc.tensor.matmul(
                                    bk["zps"][ci][:, :cw], ones[:, :],
                                    bk["zacc_b"][:, ci * CW: ci * CW + cw],
                                    start=False, stop=True)
                        if last:
                            # 1/Z: psum -> sbuf copy, K=1 PE broadcast matmul
                            # into the recycled psC bank, then reciprocal.
                            # (gpsimd partition_broadcast would thrash the Q7
                            # ucode lib against tensor_tensor: ~7us/switch.)
                            bk["zr2"] = []
                            for ci, (c0, cw) in enumerate(pair):
                                zsb = zP.tile([1, CW], bf16, name="zsb", tag="zsb", bufs=2)
                                nc.vector.tensor_copy(zsb[:, :cw], bk["zps"][ci][:, :cw])
                                zrep = psC.tile([128, CW], f32, name="zrep", tag="z", bufs=2)
                                nc.tensor.matmul(zrep[:, :cw],
                                                 onesrow[0:1, :],
                                                 zsb[0:1, :cw],
                                                 start=True, stop=True)
                                zr2 = zP.tile([128, CW], f32, name="zr2", tag="zr2", bufs=2)
                                nc.vector.reciprocal_approx_fast(out=zr2[:, :cw], in_=zrep[:, :cw])
                                bk["zr2"].append(zr2)
                        for st in sts:
                            p_t, scnt = bk["ptiles"][st]
                            for ci, (c0, cw) in enumerate(pair):
                                nc.tensor.matmul(
                                    bk["o2"][ci][:, :cw],
                                    v_sb[:scnt, st, h * HD:(h + 1) * HD],
                                    p_t[:scnt, ci * CW: ci * CW + cw],
                                    start=(st == 0), stop=(st == n_st - 1))
                        if last:
                            bk["ao"] = []
                            for ci, (c0, cw) in enumerate(pair):
                                ao_h = aoP.tile([128, CW], bf16, name="ao", tag="ao", bufs=2 * HPC + 2)
                                nc.vector.tensor_mul(ao_h[:, :cw], bk["o2"][ci][:, :cw], bk["zr2"][ci][:, :cw])
                                bk["ao"].append(ao_h)

                    def emit_outproj(pi):
                        # keep the exp stream clean: scalar only does y copies
                        # for the final pair (its exps are all done by then)
                        pair = t_pairs[pi]
                        last_pair = (pi == len(t_pairs) - 1)
                        for ci, (c0, cw) in enumerate(pair):
                            for tt in range(cw // 128):
                                y_sb = yP.tile([128, DIM], bf16, name="y_sb", tag="ysb", bufs=2)
                                for nn in range(DIM // 512):
                                    yps = psA.tile([128, 512], f32, name="yps", tag="sc", bufs=2)
                                    for h in range(HPC):
                                        nc.tensor.matmul(
                                            yps[:],
                                            blocks[(pi, h)]["ao"][ci][:, tt * 128:(tt + 1) * 128],
                                            wo[h][:, nn * 512:(nn + 1) * 512],
                                            start=(h == 0), stop=(h == HPC - 1))
                                    dst = y_sb[:, nn * 512:(nn + 1) * 512]
                                    if nn == 1:
                                        nc.vector.tensor_copy(dst, yps[:])
                                    elif last_pair:
                                        nc.scalar.copy(dst, yps[:])
                                    else:
                                        nc.vector.tensor_copy(dst, yps[:])
                                nc.sync.dma_start(out=y[c0 + tt * 128: c0 + (tt + 1) * 128, :], in_=y_sb[:])

                    # block order; sc of block i+1 interleaves with zav of block i
                    border = [(pi, h) for pi in range(len(t_pairs)) for h in range(HPC)]
                    ngr = _ceil_div(n_st, G)
                    grp = lambda bk_, g: (bk_[0], bk_[1], g * G, min(G, n_st - g * G))

                    # prime: sc for block 0 fully (exp starts early)
                    for g in range(ngr):
                        emit_sc_exp(grp(border[0], g))

                    # steady state
                    for bi in range(len(border)):
                        nxt = border[bi + 1] if bi + 1 < len(border) else None
                        for g in range(ngr):
                            if nxt is not None:
                                emit_sc_exp(grp(nxt, g))
                            emit_zav(grp(border[bi], g))
                        pi, h = border[bi]
                        if h == HPC - 1:
                            emit_outproj(pi)

    nc.compile()
    return nc


def _host_tables(T=T, S=S):
    scale = float(HD) ** (-0.25)
    inv = 1.0 / (ROPE_BASE ** (np.arange(0, HD, 2, dtype=np.float64) / HD))  # [64]

    def tables(L):
        import ml_dtypes
        fr = np.outer(inv, np.arange(L, dtype=np.float64))  # [64, L]
        c = np.cos(fr) * scale
        s = np.sin(fr) * scale
        cos = np.concatenate([c, c], axis=0).astype(ml_dtypes.bfloat16)
        sin = np.concatenate([-s, s], axis=0).astype(ml_dtypes.bfloat16)
        return np.ascontiguousarray(cos), np.ascontiguousarray(sin)

    cosq_, sinq_ = tables(T)
    cosk_, sink_ = tables(S)
    return cosq_, sinq_, cosk_, sink_


def make_in_maps(x, xmel, Wq, Wkv, Wout):
    import ml_dtypes
    bf = ml_dtypes.bfloat16
    Bx, Tx, C = x.shape
    Sx = xmel.shape[1]
    cosq_, sinq_, cosk_, sink_ = _host_tables(Tx, Sx)

    x = np.asarray(x, dtype=np.float32)
    xmel = np.asarray(xmel, dtype=np.float32)
    Wq = np.asarray(Wq, dtype=np.float32)
    Wkv = np.asarray(Wkv, dtype=np.float32)
    Wout = np.asarray(Wout, dtype=np.float32)

    xT_b = [np.ascontiguousarray(x[b].T).astype(bf) for b in range(Bx)]
    xmelT_b = [np.ascontiguousarray(xmel[b].T).astype(bf) for b in range(Bx)]
    gsz = HPC * HD  # 256
    WqT_g, WkT_g, WvT_g, WoT_g = [], [], [], []
    for g in range(NG):
        r0 = g * gsz
        def prearr(wt):  # [DIM, gsz] -> [128, NKT*gsz], row p holds [kt, n]
            return np.ascontiguousarray(
                wt.reshape(NKT, 128, gsz).transpose(1, 0, 2).reshape(128, NKT * gsz)).astype(bf)
        WqT_g.append(prearr(Wq[r0:r0 + gsz, :].T))
        WkT_g.append(prearr(Wkv[r0:r0 + gsz, :].T))
        WvT_g.append(prearr(Wkv[DIM + r0:DIM + r0 + gsz, :].T))
        WoT_g.append(np.ascontiguousarray(Wout[:, r0:r0 + gsz].T).astype(bf))

    in_maps = []
    for c in range(Bx * NG):
        b, g = c // NG, c % NG
        in_maps.append({
            "xT": xT_b[b], "xmelT": xmelT_b[b],
            "WqT": WqT_g[g], "WkT": WkT_g[g], "WvT": WvT_g[g], "WoT": WoT_g[g],
            "cosq": cosq_, "sinq": sinq_, "cosk": cosk_, "sink": sink_,
        })
    return in_maps


def kernel(x, xmel, Wq, Wkv, Wout):
    from concourse.bass_utils import run_bass_kernel_spmd

    x = np.asarray(x, dtype=np.float32)
    xmel = np.asarray(xmel, dtype=np.float32)
    Bx, Tx, C = x.shape
    Sx = xmel.shape[1]
    assert (Bx, Tx, C, Sx) == (B, T, DIM, S)

    if "nc" not in _cache:
        _cache["nc"] = build_nc()
    nc = _cache["nc"]

    in_maps = make_in_maps(x, xmel,
                           np.asarray(Wq, dtype=np.float32),
                           np.asarray(Wkv, dtype=np.float32),
                           np.asarray(Wout, dtype=np.float32))
    res = run_bass_kernel_spmd(nc, in_maps, list(range(8)))
    out = np.zeros((B, T, DIM), dtype=np.float32)
    for c in range(8):
        b = c // NG
        out[b] += res.results[c]["y"].astype(np.float32)
    return out
